# revision 6
# baseline (speedup 1.0000x reference)
"""Distributed Trainium2 kernel for nn_ApaBlock (8 NeuronCores, data-parallel).

Architecture (per core, batch shard of 256 rows):
  Z = relu(X @ W1 + b1)                              (TensorE + DVE/ACT)
  scan over 8 ranks:
    T = Zi @ P_i            64 matmuls, PSUM chunks   (TensorE, bf16)
    tmp_q = Z[:,q] * T_q    per-q scale from PSUM     (split ACT/DVE)
    G = sum_q tmp_q         identity-matmul accum     (TensorE)
    sync-BN: PE-transpose G, stats via ACT accum_out,
             cross-core AllGather (1KB), per-partition affine apply
    -> Zi+1^T directly in lhsT layout for next rank
  Y = BN(sum Zi/8); out = relu(relu(Y@W3+b3) + relu(X@W2+b2))

Inputs are sharded/preprocessed on host (free): X transposed per shard,
P flattened to (rank, p, q*k) bf16, weights bf16, biases broadcast.
"""

import os
import sys
import types

if "/opt/trn_rl_repo" not in sys.path:
    sys.path.insert(0, "/opt/trn_rl_repo")

import numpy as np
import ml_dtypes

N_CORES = 8
B, IN, H, OUT, RANK = 2048, 256, 128, 128, 8
BS = B // N_CORES  # 256 rows per core
NBT = BS // 128  # 2 b-tiles per core
EPS = 1e-5
QK = H * H  # 16384
NCHUNK = QK // 512  # 32 psum chunks per b-tile
GROUP = 4  # stage-1 chunks per identity-accum block
# fraction of per-q scale ops on DVE (rest on ACT); DVE ~258ns/unit,
# ACT ~250ns/unit -> near-even split
DVE_Q_SPLIT = 63  # q < DVE_Q_SPLIT handled by DVE, else ACT

_cache = {}


def _ensure_axon_hooks_shim():
    """bass_utils imports antenv.axon_hooks when BASS_TRACE is set; the agent
    image lacks it. Provide a null shim so tracing degrades gracefully."""
    try:
        import antenv.axon_hooks  # noqa: F401
        return
    except ImportError:
        pass
    try:
        import antenv  # noqa: F401
    except ImportError:
        return
    mod = types.ModuleType("antenv.axon_hooks")
    _state = {"hook": None}
    mod.set_axon_ntff_profile_hook = lambda h: _state.__setitem__("hook", h)
    mod.get_axon_ntff_profile_hook = lambda: _state["hook"]
    sys.modules["antenv.axon_hooks"] = mod


def _build():
    from concourse import bacc, mybir, tile

    f32 = mybir.dt.float32
    bf16 = mybir.dt.bfloat16
    FT = mybir.ActivationFunctionType
    AL = mybir.AluOpType

    nc = bacc.Bacc("TRN2", target_bir_lowering=False, debug=False,
                   num_devices=N_CORES)

    XTd = nc.declare_dram_parameter("XT", [2, 128, BS], bf16, isOutput=False)
    Pd = nc.declare_dram_parameter("P", [RANK, H, QK], bf16, isOutput=False)
    W1d = nc.declare_dram_parameter("W1", [2, 128, H], bf16, isOutput=False)
    W2d = nc.declare_dram_parameter("W2", [2, 128, OUT], bf16, isOutput=False)
    W3d = nc.declare_dram_parameter("W3", [H, OUT], bf16, isOutput=False)
    B1d = nc.declare_dram_parameter("b1b", [128, H], f32, isOutput=False)
    B2d = nc.declare_dram_parameter("b2b", [128, OUT], f32, isOutput=False)
    B3d = nc.declare_dram_parameter("b3b", [128, OUT], f32, isOutput=False)
    BNd = nc.declare_dram_parameter("bn", [H, 4], f32, isOutput=False)
    IDd = nc.declare_dram_parameter("ident", [128, 128], bf16, isOutput=False)
    OUTd = nc.declare_dram_parameter("out", [BS, OUT], f32, isOutput=True)

    rg = [list(range(N_CORES))]

    with tile.TileContext(nc) as tc:
        with (
            tc.tile_pool(name="const", bufs=1) as cpool,
            tc.tile_pool(name="ppool", bufs=2) as ppool,
            tc.tile_pool(name="tmp", bufs=2) as tmpool,
            tc.tile_pool(name="zit", bufs=2) as zitpool,
            tc.tile_pool(name="small", bufs=4) as spool,
            tc.tile_pool(name="psmm", bufs=GROUP, space="PSUM") as psmm,
            tc.tile_pool(name="psacc", bufs=1, space="PSUM") as psacc,
            tc.tile_pool(name="pstr", bufs=2, space="PSUM") as pstr,
            tc.tile_pool(name="psaux", bufs=1, space="PSUM") as psaux,
            tc.tile_pool(name="dram", bufs=4, space="DRAM") as dpool,
        ):
            # ---------------- constants ----------------
            xt = cpool.tile([128, 2 * BS], bf16, tag="xt")
            for c in range(2):
                nc.sync.dma_start(xt[:, c * BS:(c + 1) * BS], XTd[c])
            w1 = cpool.tile([128, 2 * H], bf16, tag="w1")
            w2 = cpool.tile([128, 2 * OUT], bf16, tag="w2")
            for c in range(2):
                nc.sync.dma_start(w1[:, c * H:(c + 1) * H], W1d[c])
                nc.sync.dma_start(w2[:, c * OUT:(c + 1) * OUT], W2d[c])
            w3 = cpool.tile([H, OUT], bf16, tag="w3")
            nc.sync.dma_start(w3[:], W3d[:])
            b1b = cpool.tile([128, H], f32, tag="b1b")
            b2b = cpool.tile([128, OUT], f32, tag="b2b")
            b3b = cpool.tile([128, OUT], f32, tag="b3b")
            nc.sync.dma_start(b1b[:], B1d[:])
            nc.sync.dma_start(b2b[:], B2d[:])
            nc.sync.dma_start(b3b[:], B3d[:])
            bn = cpool.tile([H, 4], f32, tag="bn")
            nc.sync.dma_start(bn[:], BNd[:])
            ident = cpool.tile([128, 128], bf16, tag="ident")
            nc.sync.dma_start(ident[:], IDd[:])

            zf = cpool.tile([128, 2 * H], f32, tag="zf")    # Z, b-partition
            zb = cpool.tile([128, 2 * H], bf16, tag="zb")
            yt = cpool.tile([H, BS], f32, tag="yt")         # Y^T accumulator
            nc.vector.memset(yt[:], 0.0)

            # ---------------- Z = relu(X@W1 + b1) ----------------
            for bt in range(NBT):
                ps = psaux.tile([128, 512], f32, tag="aux")
                for c in range(2):
                    nc.tensor.matmul(
                        ps[:, :H],
                        lhsT=xt[:, c * BS + bt * 128: c * BS + (bt + 1) * 128],
                        rhs=w1[:, c * H:(c + 1) * H],
                        start=(c == 0), stop=(c == 1),
                    )
                t0 = spool.tile([128, H], f32, tag="ztmp")
                nc.vector.tensor_tensor(t0[:], ps[:, :H], b1b[:], AL.add)
                nc.scalar.activation(zf[:, bt * H:(bt + 1) * H], t0[:], FT.Relu)
                nc.vector.tensor_copy(zb[:, bt * H:(bt + 1) * H],
                                      zf[:, bt * H:(bt + 1) * H])

            # Z^T (q-part, b) = initial Zi^T
            zit = zitpool.tile([H, BS], bf16, tag="zit")
            for bt in range(NBT):
                pst = pstr.tile([128, 128], bf16, tag="tr")
                nc.tensor.transpose(pst[:],
                                    zb[:, bt * H:(bt + 1) * H], ident[:])
                nc.scalar.activation(zit[:, bt * 128:(bt + 1) * 128],
                                     pst[:], FT.Copy)

            # ---------------- scan over ranks ----------------
            for r in range(RANK):
                p_sb = ppool.tile([128, QK], bf16, tag="p")
                nc.sync.dma_start(p_sb[:], Pd[r])

                gbf = spool.tile([128, NBT * H], bf16, tag="gbf")
                for bt in range(NBT):
                    tmp = tmpool.tile([128, QK], bf16, tag="tmp")
                    acc = psacc.tile([128, 128], f32, tag="acc")
                    lhs = zit[:, bt * 128:(bt + 1) * 128]
                    ngroups = NCHUNK // GROUP
                    chunk_ps = {}
                    for g in range(ngroups):
                        # stage-1 matmuls for this group
                        for ci in range(GROUP):
                            c = g * GROUP + ci
                            ps = psmm.tile([128, 512], f32, tag="mm")
                            chunk_ps[c] = ps
                            nc.tensor.matmul(ps[:], lhsT=lhs,
                                             rhs=p_sb[:, c * 512:(c + 1) * 512],
                                             start=True, stop=True)
                        # per-q scale: tmp_q = Z[:, q] * T_q
                        for ci in range(GROUP):
                            c = g * GROUP + ci
                            ps = chunk_ps[c]
                            for j in range(4):
                                q = c * 4 + j
                                dst = tmp[:, q * 128:(q + 1) * 128]
                                src = ps[:, j * 128:(j + 1) * 128]
                                scal = zf[:, bt * H + q: bt * H + q + 1]
                                if q < DVE_Q_SPLIT:
                                    nc.vector.tensor_scalar(
                                        dst, src, scal, None, AL.mult)
                                else:
                                    nc.scalar.activation(
                                        dst, src, FT.Copy, scale=scal)
                        # identity-matmul accumulation of this group's planes
                        for ci in range(GROUP):
                            c = g * GROUP + ci
                            for j in range(4):
                                q = c * 4 + j
                                nc.tensor.matmul(
                                    acc[:], lhsT=ident[:],
                                    rhs=tmp[:, q * 128:(q + 1) * 128],
                                    start=(q == 0), stop=(q == QK // 128 - 1),
                                )
                    # G (b-part) -> bf16 for transpose
                    nc.scalar.activation(gbf[:, bt * H:(bt + 1) * H],
                                         acc[:], FT.Copy)

                # transpose G -> (k, b), evac + batch stats via accum_out
                gt = spool.tile([H, BS], bf16, tag="gt")
                scr = spool.tile([128, 128], bf16, tag="scr")
                s1 = spool.tile([H, 8], f32, tag="stat")
                for bt in range(NBT):
                    pst = pstr.tile([128, 128], bf16, tag="tr")
                    nc.tensor.transpose(pst[:],
                                        gbf[:, bt * H:(bt + 1) * H], ident[:])
                    nc.scalar.activation(gt[:, bt * 128:(bt + 1) * 128],
                                         pst[:], FT.Copy,
                                         accum_out=s1[:, bt:bt + 1])
                    nc.scalar.activation(scr[:], pst[:], FT.Square,
                                         accum_out=s1[:, 2 + bt:3 + bt])
                stl = spool.tile([H, 2], f32, tag="stl")
                nc.vector.tensor_tensor(stl[:, 0:1], s1[:, 0:1], s1[:, 1:2],
                                        AL.add)
                nc.vector.tensor_tensor(stl[:, 1:2], s1[:, 2:3], s1[:, 3:4],
                                        AL.add)

                # ---- cross-core AllGather of (H, 2) stats ----
                a_ap, c_ap = _bn_sync(nc, tc, dpool, spool, stl, bn,
                                      gcol=0, bcol=1, extra_scale=None)

                # apply BN + produce next Zi^T; accumulate Y^T
                zit_next = zitpool.tile([H, BS], bf16, tag="zit")
                nc.vector.tensor_scalar(zit_next[:], gt[:], a_ap, c_ap,
                                        AL.mult, AL.add)
                nc.vector.tensor_tensor(yt[:], yt[:], zit_next[:], AL.add)
                zit = zit_next

            # ---------------- Y BN (on Y/8 via stats scale trick) ----------
            sy = spool.tile([H, 8], f32, tag="stat")
            scr2 = spool.tile([H, BS], bf16, tag="scry")
            nc.scalar.activation(scr2[:], yt[:], FT.Copy, scale=0.125,
                                 accum_out=sy[:, 0:1])
            nc.scalar.activation(scr2[:], yt[:], FT.Square, scale=0.125,
                                 accum_out=sy[:, 1:2])
            styl = spool.tile([H, 2], f32, tag="stl")
            nc.vector.tensor_copy(styl[:], sy[:, 0:2])
            ay_ap, cy_ap = _bn_sync(nc, tc, dpool, spool, styl, bn,
                                    gcol=2, bcol=3, extra_scale=0.125)
            ybn = spool.tile([H, BS], bf16, tag="ybn")
            nc.vector.tensor_scalar(ybn[:], yt[:], ay_ap, cy_ap,
                                    AL.mult, AL.add)

            # ---------------- final: relu(relu(Y@W3+b3)+relu(X@W2+b2)) ----
            for bt in range(NBT):
                psA = psaux.tile([128, 512], f32, tag="aux")
                nc.tensor.matmul(psA[:, :OUT],
                                 lhsT=ybn[:, bt * 128:(bt + 1) * 128],
                                 rhs=w3[:], start=True, stop=True)
                r1 = spool.tile([128, OUT], f32, tag="r1")
                nc.vector.tensor_tensor(r1[:], psA[:, :OUT], b3b[:], AL.add)
                r1r = spool.tile([128, OUT], f32, tag="r1r")
                nc.scalar.activation(r1r[:], r1[:], FT.Relu)

                psB = psaux.tile([128, 512], f32, tag="aux")
                for c in range(2):
                    nc.tensor.matmul(
                        psB[:, :OUT],
                        lhsT=xt[:, c * BS + bt * 128: c * BS + (bt + 1) * 128],
                        rhs=w2[:, c * OUT:(c + 1) * OUT],
                        start=(c == 0), stop=(c == 1),
                    )
                r2 = spool.tile([128, OUT], f32, tag="r2")
                nc.vector.tensor_tensor(r2[:], psB[:, :OUT], b2b[:], AL.add)
                r2r = spool.tile([128, OUT], f32, tag="r2r")
                nc.scalar.activation(r2r[:], r2[:], FT.Relu)

                s = spool.tile([128, OUT], f32, tag="s")
                nc.vector.tensor_tensor(s[:], r1r[:], r2r[:], AL.add)
                of = spool.tile([128, OUT], f32, tag="of")
                nc.scalar.activation(of[:], s[:], FT.Relu)
                nc.sync.dma_start(OUTd[bt * 128:(bt + 1) * 128, :], of[:])

    nc.compile()
    return nc


def _bn_sync(nc, tc, dpool, spool, stl, bn, gcol, bcol, extra_scale):
    """AllGather per-core (H,2) [sum, sumsq] stats, reduce across 8 cores,
    compute affine coeffs a, c s.t. BN(x) = a*x + c (per-partition).

    If extra_scale is set, stats were computed on (extra_scale*x) and the
    returned a is pre-multiplied by extra_scale so a*x + c uses raw x.
    """
    from concourse import mybir

    f32 = mybir.dt.float32
    FT = mybir.ActivationFunctionType
    AL = mybir.AluOpType

    src = dpool.tile([H, 2], f32, tag="ccsrc")
    dst = dpool.tile([N_CORES * H, 2], f32, tag="ccdst")
    nc.sync.dma_start(src[:], stl[:])
    nc.gpsimd.collective_compute(
        "AllGather", AL.bypass, replica_groups=[list(range(N_CORES))],
        ins=[src.opt()], outs=[dst.opt()],
    )
    gath = spool.tile([H, 16], f32, tag="gath")
    nc.sync.dma_start(
        gath[:].rearrange("k (c s) -> k c s", c=N_CORES),
        dst[:].rearrange("(c k) s -> k c s", c=N_CORES))
    # reduce over cores: layout (k, (c, s)) c-major pairs
    r4 = spool.tile([H, 8], f32, tag="r4")
    nc.vector.tensor_tensor(r4[:], gath[:, 0:8], gath[:, 8:16], AL.add)
    r2 = spool.tile([H, 4], f32, tag="r2s")
    nc.vector.tensor_tensor(r2[:], r4[:, 0:4], r4[:, 4:8], AL.add)
    st = spool.tile([H, 2], f32, tag="stg")
    nc.vector.tensor_tensor(st[:], r2[:, 0:2], r2[:, 2:4], AL.add)

    cf = spool.tile([H, 8], f32, tag="cf")
    m = cf[:, 0:1]
    ex2 = cf[:, 1:2]
    v = cf[:, 2:3]
    sd = cf[:, 3:4]
    rinv = cf[:, 4:5]
    a = cf[:, 5:6]
    t = cf[:, 6:7]
    c = cf[:, 7:8]
    nc.vector.tensor_scalar(m, st[:, 0:1], 1.0 / B, None, AL.mult)
    nc.vector.tensor_scalar(ex2, st[:, 1:2], 1.0 / B, None, AL.mult)
    msq = spool.tile([H, 1], f32, tag="msq")
    nc.vector.tensor_tensor(msq[:], m, m, AL.mult)
    nc.vector.tensor_tensor(v, ex2, msq[:], AL.subtract)
    nc.vector.tensor_scalar(v, v, EPS, None, AL.add)
    nc.scalar.activation(sd, v, FT.Sqrt)
    nc.vector.reciprocal(rinv, sd)
    nc.vector.tensor_tensor(a, rinv, bn[:, gcol:gcol + 1], AL.mult)
    nc.vector.tensor_tensor(t, m, a, AL.mult)
    nc.vector.tensor_tensor(c, bn[:, bcol:bcol + 1], t, AL.subtract)
    if extra_scale is not None:
        a_out = cf[:, 4:5]  # reuse rinv slot
        nc.vector.tensor_scalar(a_out, a, extra_scale, None, AL.mult)
        return a_out, c
    return a, c


def _prep_inputs(X, W1, b1, W2, b2, W3, b3, P, gz, bz, gy, by):
    bf = ml_dtypes.bfloat16
    per_core = []
    P_b = np.ascontiguousarray(P.reshape(RANK, H, QK)).astype(bf)
    W1_b = np.ascontiguousarray(W1.reshape(2, 128, H)).astype(bf)
    W2_b = np.ascontiguousarray(W2.reshape(2, 128, OUT)).astype(bf)
    W3_b = np.ascontiguousarray(W3).astype(bf)
    b1b = np.broadcast_to(b1, (128, H)).astype(np.float32).copy()
    b2b = np.broadcast_to(b2, (128, OUT)).astype(np.float32).copy()
    b3b = np.broadcast_to(b3, (128, OUT)).astype(np.float32).copy()
    bnc = np.stack([gz, bz, gy, by], axis=1).astype(np.float32)
    ident = np.eye(128, dtype=np.float32).astype(bf)
    for s in range(N_CORES):
        Xs = X[s * BS:(s + 1) * BS]
        XT = np.ascontiguousarray(Xs.T.reshape(2, 128, BS)).astype(bf)
        per_core.append({
            "XT": XT, "P": P_b, "W1": W1_b, "W2": W2_b, "W3": W3_b,
            "b1b": b1b, "b2b": b2b, "b3b": b3b, "bn": bnc, "ident": ident,
        })
    return per_core


def kernel(**inputs):
    _ensure_axon_hooks_shim()
    from concourse.bass_utils import run_bass_kernel_spmd

    if "nc" not in _cache:
        _cache["nc"] = _build()
    nc = _cache["nc"]

    in_maps = _prep_inputs(**{k: np.asarray(v) for k, v in inputs.items()})
    res = run_bass_kernel_spmd(nc, in_maps, core_ids=list(range(N_CORES)))
    out = np.concatenate([m["out"] for m in res.results], axis=0)
    return out.astype(np.float32)


if __name__ == "__main__":
    import reference as R

    inputs = {k: np.asarray(v) for k, v in R.setup_inputs().items()}
    got = kernel(**inputs)
    exp = np.asarray(R.reference(**R.setup_inputs()))
    rel = np.linalg.norm(got - exp) / np.linalg.norm(exp)
    print("rel l2:", rel)


# revision 13
# speedup vs baseline: 1.6699x; 1.6699x over previous
"""Distributed Trainium2 kernel for nn_ApaBlock (8 NeuronCores, data-parallel).

Architecture (per core, batch shard of 256 rows):
  Z = relu(X @ W1 + b1)                              (TensorE + DVE/ACT)
  scan over 8 ranks:
    T = Zi @ P_i            64 matmuls, PSUM chunks   (TensorE, bf16)
    tmp_q = Z[:,q] * T_q    per-q scale from PSUM     (split ACT/DVE)
    G = sum_q tmp_q         identity-matmul accum     (TensorE)
    sync-BN: PE-transpose G, stats via ACT accum_out,
             cross-core AllGather (1KB), per-partition affine apply
    -> Zi+1^T directly in lhsT layout for next rank
  Y = BN(sum Zi/8); out = relu(relu(Y@W3+b3) + relu(X@W2+b2))

Inputs are sharded/preprocessed on host (free): X transposed per shard,
P flattened to (rank, p, q*k) bf16, weights bf16, biases broadcast.
"""

import os
import sys
import types

if "/opt/trn_rl_repo" not in sys.path:
    sys.path.insert(0, "/opt/trn_rl_repo")

import numpy as np
import ml_dtypes

N_CORES = 8
B, IN, H, OUT, RANK = 2048, 256, 128, 128, 8
BS = B // N_CORES  # 256 rows per core
NBT = BS // 128  # 2 b-tiles per core
EPS = 1e-5
QK = H * H  # 16384
MACRO = 1024  # psum macro-chunk width (8 q-planes, 2 matmuls)
NCHUNK = QK // MACRO  # 16 macro-chunks per b-tile
QPM = MACRO // H  # q-planes per macro-chunk (8)

_cache = {}


def _ensure_axon_hooks_shim():
    """bass_utils imports antenv.axon_hooks when BASS_TRACE is set; the agent
    image lacks it. Provide a null shim so tracing degrades gracefully."""
    try:
        import antenv.axon_hooks  # noqa: F401
        return
    except ImportError:
        pass
    try:
        import antenv  # noqa: F401
    except ImportError:
        return
    mod = types.ModuleType("antenv.axon_hooks")
    _state = {"hook": None}
    mod.set_axon_ntff_profile_hook = lambda h: _state.__setitem__("hook", h)
    mod.get_axon_ntff_profile_hook = lambda: _state["hook"]
    sys.modules["antenv.axon_hooks"] = mod


def _build():
    from concourse import bacc, mybir, tile

    f32 = mybir.dt.float32
    bf16 = mybir.dt.bfloat16
    FT = mybir.ActivationFunctionType
    AL = mybir.AluOpType

    nc = bacc.Bacc("TRN2", target_bir_lowering=False, debug=False,
                   num_devices=N_CORES)

    XTd = nc.declare_dram_parameter("XT", [2, 128, BS], bf16, isOutput=False)
    Pd = nc.declare_dram_parameter("P", [RANK, H, QK], bf16, isOutput=False)
    W1d = nc.declare_dram_parameter("W1", [2, 128, H], bf16, isOutput=False)
    W2d = nc.declare_dram_parameter("W2", [2, 128, OUT], bf16, isOutput=False)
    W3d = nc.declare_dram_parameter("W3", [H, OUT], bf16, isOutput=False)
    B1d = nc.declare_dram_parameter("b1b", [128, H], f32, isOutput=False)
    B2d = nc.declare_dram_parameter("b2b", [128, OUT], f32, isOutput=False)
    B3d = nc.declare_dram_parameter("b3b", [128, OUT], f32, isOutput=False)
    BNd = nc.declare_dram_parameter("bn", [H, 4], f32, isOutput=False)
    IDd = nc.declare_dram_parameter("ident", [128, 128], bf16, isOutput=False)
    OUTd = nc.declare_dram_parameter("out", [BS, OUT], f32, isOutput=True)

    rg = [list(range(N_CORES))]

    with tile.TileContext(nc) as tc:
        with (
            tc.tile_pool(name="const", bufs=1) as cpool,
            tc.tile_pool(name="ppool", bufs=2) as ppool,
            tc.tile_pool(name="tmp", bufs=2) as tmpool,
            tc.tile_pool(name="zit", bufs=2) as zitpool,
            tc.tile_pool(name="small", bufs=4) as spool,
            tc.tile_pool(name="psmm", bufs=3, space="PSUM") as psmm,
            tc.tile_pool(name="psacc", bufs=1, space="PSUM") as psacc,
            tc.tile_pool(name="pstr", bufs=1, space="PSUM") as pstr,
            tc.tile_pool(name="dram", bufs=4, space="DRAM") as dpool,
        ):
            # ---------------- constants ----------------
            xt = cpool.tile([128, 2 * BS], bf16, tag="xt")
            for c in range(2):
                nc.sync.dma_start(xt[:, c * BS:(c + 1) * BS], XTd[c])
            w1 = cpool.tile([128, 2 * H], bf16, tag="w1")
            w2 = cpool.tile([128, 2 * OUT], bf16, tag="w2")
            for c in range(2):
                nc.sync.dma_start(w1[:, c * H:(c + 1) * H], W1d[c])
                nc.sync.dma_start(w2[:, c * OUT:(c + 1) * OUT], W2d[c])
            w3 = cpool.tile([H, OUT], bf16, tag="w3")
            nc.sync.dma_start(w3[:], W3d[:])
            b1b = cpool.tile([128, H], f32, tag="b1b")
            b2b = cpool.tile([128, OUT], f32, tag="b2b")
            b3b = cpool.tile([128, OUT], f32, tag="b3b")
            nc.sync.dma_start(b1b[:], B1d[:])
            nc.sync.dma_start(b2b[:], B2d[:])
            nc.sync.dma_start(b3b[:], B3d[:])
            bn = cpool.tile([H, 4], f32, tag="bn")
            nc.sync.dma_start(bn[:], BNd[:])
            ident = cpool.tile([128, 128], bf16, tag="ident")
            nc.sync.dma_start(ident[:], IDd[:])

            zf = cpool.tile([128, 2 * H], f32, tag="zf")    # Z, b-partition
            zb = cpool.tile([128, 2 * H], bf16, tag="zb")
            yt = cpool.tile([H, BS], f32, tag="yt")         # Y^T accumulator
            nc.vector.memset(yt[:], 0.0)

            # Early dummy collective: absorbs cross-core launch skew while
            # the engines do setup + rank-0 compute (collectives run on
            # TOPSP/SDMA, serialized before the first real sync).
            dsrc = dpool.tile([H, 2], f32, tag="ccsrc")
            ddst = dpool.tile([N_CORES * H, 2], f32, tag="ccdst")
            nc.sync.dma_start(dsrc[:], bn[:, 0:2])
            nc.gpsimd.collective_compute(
                "AllGather", AL.bypass, replica_groups=rg,
                ins=[dsrc.opt()], outs=[ddst.opt()],
            )

            # ---------------- Z = relu(X@W1 + b1) ----------------
            for bt in range(NBT):
                ps = psmm.tile([128, MACRO], f32, tag="mm")
                for c in range(2):
                    nc.tensor.matmul(
                        ps[:, :H],
                        lhsT=xt[:, c * BS + bt * 128: c * BS + (bt + 1) * 128],
                        rhs=w1[:, c * H:(c + 1) * H],
                        start=(c == 0), stop=(c == 1),
                    )
                t0 = spool.tile([128, H], f32, tag="ztmp")
                nc.vector.tensor_tensor(t0[:], ps[:, :H], b1b[:], AL.add)
                nc.scalar.activation(zf[:, bt * H:(bt + 1) * H], t0[:], FT.Relu)
                nc.vector.tensor_copy(zb[:, bt * H:(bt + 1) * H],
                                      zf[:, bt * H:(bt + 1) * H])

            # Z^T (q-part, b) = initial Zi^T
            zit = zitpool.tile([H, BS], bf16, tag="zit")
            for bt in range(NBT):
                pst = pstr.tile([128, 128], bf16, tag="tr")
                nc.tensor.transpose(pst[:],
                                    zb[:, bt * H:(bt + 1) * H], ident[:])
                nc.scalar.activation(zit[:, bt * 128:(bt + 1) * 128],
                                     pst[:], FT.Copy)

            # ---------------- scan over ranks ----------------
            for r in range(RANK):
                p_sb = ppool.tile([128, QK], bf16, tag="p")
                nc.sync.dma_start(p_sb[:], Pd[r])

                gbf = spool.tile([128, NBT * H], bf16, tag="gbf")
                for bt in range(NBT):
                    tmp = tmpool.tile([128, QK], bf16, tag="tmp")
                    acc = psacc.tile([128, 128], f32, tag="acc")
                    lhs = zit[:, bt * 128:(bt + 1) * 128]
                    for c in range(NCHUNK):
                        # stage-1 matmuls: two 512-wide into one macro psum
                        ps = psmm.tile([128, MACRO], f32, tag="mm")
                        for h in range(MACRO // 512):
                            nc.tensor.matmul(
                                ps[:, h * 512:(h + 1) * 512], lhsT=lhs,
                                rhs=p_sb[:, c * MACRO + h * 512:
                                         c * MACRO + (h + 1) * 512],
                                start=True, stop=True)
                        # scale all QPM q-planes in one DVE op:
                        # tmp[b, q, k] = psum[b, q, k] * Z[b, q]
                        zsl = zf[:, bt * H + c * QPM: bt * H + (c + 1) * QPM]
                        nc.vector.tensor_tensor(
                            tmp[:, c * MACRO:(c + 1) * MACRO].rearrange(
                                "p (a b) -> p a b", b=H),
                            ps[:].rearrange("p (a b) -> p a b", b=H),
                            zsl.broadcast_to((128, QPM, H)),
                            AL.mult)
                        # identity-matmul accumulation of the scaled planes
                        for j in range(QPM):
                            q = c * QPM + j
                            nc.tensor.matmul(
                                acc[:], lhsT=ident[:],
                                rhs=tmp[:, q * 128:(q + 1) * 128],
                                start=(q == 0), stop=(q == QK // 128 - 1),
                            )
                    # G (b-part) -> bf16 for transpose
                    nc.scalar.activation(gbf[:, bt * H:(bt + 1) * H],
                                         acc[:], FT.Copy)

                # transpose G -> (k, b), evac + batch stats via accum_out
                gt = spool.tile([H, BS], bf16, tag="gt")
                scr = spool.tile([128, 128], bf16, tag="scr")
                s1 = spool.tile([H, 8], f32, tag="stat")
                for bt in range(NBT):
                    pst = pstr.tile([128, 128], bf16, tag="tr")
                    nc.tensor.transpose(pst[:],
                                        gbf[:, bt * H:(bt + 1) * H], ident[:])
                    nc.scalar.activation(gt[:, bt * 128:(bt + 1) * 128],
                                         pst[:], FT.Copy,
                                         accum_out=s1[:, bt:bt + 1])
                    nc.scalar.activation(scr[:], pst[:], FT.Square,
                                         accum_out=s1[:, 2 + bt:3 + bt])
                stl = spool.tile([H, 2], f32, tag="stl")
                nc.vector.tensor_tensor(stl[:, 0:1], s1[:, 0:1], s1[:, 1:2],
                                        AL.add)
                nc.vector.tensor_tensor(stl[:, 1:2], s1[:, 2:3], s1[:, 3:4],
                                        AL.add)

                # ---- cross-core AllGather of (H, 2) stats ----
                a_ap, c_ap = _bn_sync(nc, tc, dpool, spool, stl, bn,
                                      gcol=0, bcol=1, extra_scale=None)

                # apply BN + produce next Zi^T; accumulate Y^T
                zit_next = zitpool.tile([H, BS], bf16, tag="zit")
                nc.vector.tensor_scalar(zit_next[:], gt[:], a_ap, c_ap,
                                        AL.mult, AL.add)
                nc.vector.tensor_tensor(yt[:], yt[:], zit_next[:], AL.add)
                zit = zit_next

            # ---------------- Y BN (on Y/8 via stats scale trick) ----------
            sy = spool.tile([H, 8], f32, tag="stat")
            scr2 = spool.tile([H, BS], bf16, tag="scry")
            nc.scalar.activation(scr2[:], yt[:], FT.Copy, scale=0.125,
                                 accum_out=sy[:, 0:1])
            nc.scalar.activation(scr2[:], yt[:], FT.Square, scale=0.125,
                                 accum_out=sy[:, 1:2])
            styl = spool.tile([H, 2], f32, tag="stl")
            nc.vector.tensor_copy(styl[:], sy[:, 0:2])
            ay_ap, cy_ap = _bn_sync(nc, tc, dpool, spool, styl, bn,
                                    gcol=2, bcol=3, extra_scale=0.125)
            ybn = spool.tile([H, BS], bf16, tag="ybn")
            nc.vector.tensor_scalar(ybn[:], yt[:], ay_ap, cy_ap,
                                    AL.mult, AL.add)

            # ---------------- final: relu(relu(Y@W3+b3)+relu(X@W2+b2)) ----
            for bt in range(NBT):
                psA = psmm.tile([128, MACRO], f32, tag="mm")
                nc.tensor.matmul(psA[:, :OUT],
                                 lhsT=ybn[:, bt * 128:(bt + 1) * 128],
                                 rhs=w3[:], start=True, stop=True)
                r1 = spool.tile([128, OUT], f32, tag="r1")
                nc.vector.tensor_tensor(r1[:], psA[:, :OUT], b3b[:], AL.add)
                r1r = spool.tile([128, OUT], f32, tag="r1r")
                nc.scalar.activation(r1r[:], r1[:], FT.Relu)

                psB = psmm.tile([128, MACRO], f32, tag="mm")
                for c in range(2):
                    nc.tensor.matmul(
                        psB[:, :OUT],
                        lhsT=xt[:, c * BS + bt * 128: c * BS + (bt + 1) * 128],
                        rhs=w2[:, c * OUT:(c + 1) * OUT],
                        start=(c == 0), stop=(c == 1),
                    )
                r2 = spool.tile([128, OUT], f32, tag="r2")
                nc.vector.tensor_tensor(r2[:], psB[:, :OUT], b2b[:], AL.add)
                r2r = spool.tile([128, OUT], f32, tag="r2r")
                nc.scalar.activation(r2r[:], r2[:], FT.Relu)

                s = spool.tile([128, OUT], f32, tag="s")
                nc.vector.tensor_tensor(s[:], r1r[:], r2r[:], AL.add)
                of = spool.tile([128, OUT], f32, tag="of")
                nc.scalar.activation(of[:], s[:], FT.Relu)
                nc.sync.dma_start(OUTd[bt * 128:(bt + 1) * 128, :], of[:])

    nc.compile()
    return nc


def _bn_sync(nc, tc, dpool, spool, stl, bn, gcol, bcol, extra_scale):
    """AllGather per-core (H,2) [sum, sumsq] stats, reduce across 8 cores,
    compute affine coeffs a, c s.t. BN(x) = a*x + c (per-partition).

    If extra_scale is set, stats were computed on (extra_scale*x) and the
    returned a is pre-multiplied by extra_scale so a*x + c uses raw x.
    """
    from concourse import mybir

    f32 = mybir.dt.float32
    FT = mybir.ActivationFunctionType
    AL = mybir.AluOpType

    src = dpool.tile([H, 2], f32, tag="ccsrc")
    dst = dpool.tile([N_CORES * H, 2], f32, tag="ccdst")
    nc.sync.dma_start(src[:], stl[:])
    nc.gpsimd.collective_compute(
        "AllGather", AL.bypass, replica_groups=[list(range(N_CORES))],
        ins=[src.opt()], outs=[dst.opt()],
    )
    gath = spool.tile([H, 16], f32, tag="gath")
    nc.sync.dma_start(
        gath[:].rearrange("k (c s) -> k c s", c=N_CORES),
        dst[:].rearrange("(c k) s -> k c s", c=N_CORES))
    # reduce over cores: layout (k, (c, s)) c-major pairs
    r4 = spool.tile([H, 8], f32, tag="r4")
    nc.vector.tensor_tensor(r4[:], gath[:, 0:8], gath[:, 8:16], AL.add)
    r2 = spool.tile([H, 4], f32, tag="r2s")
    nc.vector.tensor_tensor(r2[:], r4[:, 0:4], r4[:, 4:8], AL.add)
    st = spool.tile([H, 2], f32, tag="stg")
    nc.vector.tensor_tensor(st[:], r2[:, 0:2], r2[:, 2:4], AL.add)

    cf = spool.tile([H, 8], f32, tag="cf")
    m = cf[:, 0:1]
    ex2 = cf[:, 1:2]
    v = cf[:, 2:3]
    sd = cf[:, 3:4]
    rinv = cf[:, 4:5]
    a = cf[:, 5:6]
    t = cf[:, 6:7]
    c = cf[:, 7:8]
    nc.vector.tensor_scalar(m, st[:, 0:1], 1.0 / B, None, AL.mult)
    nc.vector.tensor_scalar(ex2, st[:, 1:2], 1.0 / B, None, AL.mult)
    msq = spool.tile([H, 1], f32, tag="msq")
    nc.vector.tensor_tensor(msq[:], m, m, AL.mult)
    nc.vector.tensor_tensor(v, ex2, msq[:], AL.subtract)
    nc.vector.tensor_scalar(v, v, EPS, None, AL.add)
    nc.scalar.activation(sd, v, FT.Sqrt)
    nc.vector.reciprocal(rinv, sd)
    nc.vector.tensor_tensor(a, rinv, bn[:, gcol:gcol + 1], AL.mult)
    nc.vector.tensor_tensor(t, m, a, AL.mult)
    nc.vector.tensor_tensor(c, bn[:, bcol:bcol + 1], t, AL.subtract)
    if extra_scale is not None:
        a_out = cf[:, 4:5]  # reuse rinv slot
        nc.vector.tensor_scalar(a_out, a, extra_scale, None, AL.mult)
        return a_out, c
    return a, c


def _prep_inputs(X, W1, b1, W2, b2, W3, b3, P, gz, bz, gy, by):
    bf = ml_dtypes.bfloat16
    per_core = []
    P_b = np.ascontiguousarray(P.reshape(RANK, H, QK)).astype(bf)
    W1_b = np.ascontiguousarray(W1.reshape(2, 128, H)).astype(bf)
    W2_b = np.ascontiguousarray(W2.reshape(2, 128, OUT)).astype(bf)
    W3_b = np.ascontiguousarray(W3).astype(bf)
    b1b = np.broadcast_to(b1, (128, H)).astype(np.float32).copy()
    b2b = np.broadcast_to(b2, (128, OUT)).astype(np.float32).copy()
    b3b = np.broadcast_to(b3, (128, OUT)).astype(np.float32).copy()
    bnc = np.stack([gz, bz, gy, by], axis=1).astype(np.float32)
    ident = np.eye(128, dtype=np.float32).astype(bf)
    for s in range(N_CORES):
        Xs = X[s * BS:(s + 1) * BS]
        XT = np.ascontiguousarray(Xs.T.reshape(2, 128, BS)).astype(bf)
        per_core.append({
            "XT": XT, "P": P_b, "W1": W1_b, "W2": W2_b, "W3": W3_b,
            "b1b": b1b, "b2b": b2b, "b3b": b3b, "bn": bnc, "ident": ident,
        })
    return per_core


def kernel(**inputs):
    _ensure_axon_hooks_shim()
    from concourse.bass_utils import run_bass_kernel_spmd

    if "nc" not in _cache:
        _cache["nc"] = _build()
    nc = _cache["nc"]

    in_maps = _prep_inputs(**{k: np.asarray(v) for k, v in inputs.items()})
    res = run_bass_kernel_spmd(nc, in_maps, core_ids=list(range(N_CORES)))
    out = np.concatenate([m["out"] for m in res.results], axis=0)
    return out.astype(np.float32)


if __name__ == "__main__":
    import reference as R

    inputs = {k: np.asarray(v) for k, v in R.setup_inputs().items()}
    got = kernel(**inputs)
    exp = np.asarray(R.reference(**R.setup_inputs()))
    rel = np.linalg.norm(got - exp) / np.linalg.norm(exp)
    print("rel l2:", rel)


# revision 15
# speedup vs baseline: 1.6844x; 1.0087x over previous
"""Distributed Trainium2 kernel for nn_ApaBlock (8 NeuronCores, data-parallel).

Architecture (per core, batch shard of 256 rows):
  Z = relu(X @ W1 + b1)                              (TensorE + DVE/ACT)
  scan over 8 ranks:
    T = Zi @ P_i            64 matmuls, PSUM chunks   (TensorE, bf16)
    tmp_q = Z[:,q] * T_q    per-q scale from PSUM     (split ACT/DVE)
    G = sum_q tmp_q         identity-matmul accum     (TensorE)
    sync-BN: PE-transpose G, stats via ACT accum_out,
             cross-core AllGather (1KB), per-partition affine apply
    -> Zi+1^T directly in lhsT layout for next rank
  Y = BN(sum Zi/8); out = relu(relu(Y@W3+b3) + relu(X@W2+b2))

Inputs are sharded/preprocessed on host (free): X transposed per shard,
P flattened to (rank, p, q*k) bf16, weights bf16, biases broadcast.
"""

import os
import sys
import types

if "/opt/trn_rl_repo" not in sys.path:
    sys.path.insert(0, "/opt/trn_rl_repo")

import numpy as np
import ml_dtypes

N_CORES = 8
B, IN, H, OUT, RANK = 2048, 256, 128, 128, 8
BS = B // N_CORES  # 256 rows per core
NBT = BS // 128  # 2 b-tiles per core
EPS = 1e-5
QK = H * H  # 16384
MACRO = 1024  # psum macro-chunk width (8 q-planes, 2 matmuls)
NCHUNK = QK // MACRO  # 16 macro-chunks per b-tile
QPM = MACRO // H  # q-planes per macro-chunk (8)

_cache = {}


def _ensure_axon_hooks_shim():
    """bass_utils imports antenv.axon_hooks when BASS_TRACE is set; the agent
    image lacks it. Provide a null shim so tracing degrades gracefully."""
    try:
        import antenv.axon_hooks  # noqa: F401
        return
    except ImportError:
        pass
    try:
        import antenv  # noqa: F401
    except ImportError:
        return
    mod = types.ModuleType("antenv.axon_hooks")
    _state = {"hook": None}
    mod.set_axon_ntff_profile_hook = lambda h: _state.__setitem__("hook", h)
    mod.get_axon_ntff_profile_hook = lambda: _state["hook"]
    sys.modules["antenv.axon_hooks"] = mod


def _build():
    from concourse import bacc, mybir, tile

    f32 = mybir.dt.float32
    bf16 = mybir.dt.bfloat16
    FT = mybir.ActivationFunctionType
    AL = mybir.AluOpType

    nc = bacc.Bacc("TRN2", target_bir_lowering=False, debug=False,
                   num_devices=N_CORES)

    XTd = nc.declare_dram_parameter("XT", [2, 128, BS], bf16, isOutput=False)
    Pd = nc.declare_dram_parameter("P", [RANK, H, QK], bf16, isOutput=False)
    W1d = nc.declare_dram_parameter("W1", [2, 128, H], bf16, isOutput=False)
    W2d = nc.declare_dram_parameter("W2", [2, 128, OUT], bf16, isOutput=False)
    W3d = nc.declare_dram_parameter("W3", [H, OUT], bf16, isOutput=False)
    B1d = nc.declare_dram_parameter("b1b", [128, H], f32, isOutput=False)
    B2d = nc.declare_dram_parameter("b2b", [128, OUT], f32, isOutput=False)
    B3d = nc.declare_dram_parameter("b3b", [128, OUT], f32, isOutput=False)
    BNd = nc.declare_dram_parameter("bn", [H, 4], f32, isOutput=False)
    IDd = nc.declare_dram_parameter("ident", [128, 128], bf16, isOutput=False)
    OUTd = nc.declare_dram_parameter("out", [BS, OUT], f32, isOutput=True)

    rg = [list(range(N_CORES))]

    with tile.TileContext(nc) as tc:
        with (
            tc.tile_pool(name="const", bufs=1) as cpool,
            tc.tile_pool(name="ppool", bufs=2) as ppool,
            tc.tile_pool(name="tmp", bufs=2) as tmpool,
            tc.tile_pool(name="zit", bufs=2) as zitpool,
            tc.tile_pool(name="small", bufs=4) as spool,
            tc.tile_pool(name="psmm", bufs=3, space="PSUM") as psmm,
            tc.tile_pool(name="psacc", bufs=1, space="PSUM") as psacc,
            tc.tile_pool(name="pstr", bufs=1, space="PSUM") as pstr,
            tc.tile_pool(name="dram", bufs=4, space="DRAM") as dpool,
        ):
            # ---------------- constants ----------------
            xt = cpool.tile([128, 2 * BS], bf16, tag="xt")
            for c in range(2):
                nc.sync.dma_start(xt[:, c * BS:(c + 1) * BS], XTd[c])
            w1 = cpool.tile([128, 2 * H], bf16, tag="w1")
            w2 = cpool.tile([128, 2 * OUT], bf16, tag="w2")
            for c in range(2):
                nc.sync.dma_start(w1[:, c * H:(c + 1) * H], W1d[c])
                nc.sync.dma_start(w2[:, c * OUT:(c + 1) * OUT], W2d[c])
            w3 = cpool.tile([H, OUT], bf16, tag="w3")
            nc.sync.dma_start(w3[:], W3d[:])
            b1b = cpool.tile([128, H], f32, tag="b1b")
            b2b = cpool.tile([128, OUT], f32, tag="b2b")
            b3b = cpool.tile([128, OUT], f32, tag="b3b")
            nc.sync.dma_start(b1b[:], B1d[:])
            nc.sync.dma_start(b2b[:], B2d[:])
            nc.sync.dma_start(b3b[:], B3d[:])
            bn = cpool.tile([H, 4], f32, tag="bn")
            nc.sync.dma_start(bn[:], BNd[:])
            ident = cpool.tile([128, 128], bf16, tag="ident")
            nc.sync.dma_start(ident[:], IDd[:])

            zf = cpool.tile([128, 2 * H], f32, tag="zf")    # Z, b-partition
            zb = cpool.tile([128, 2 * H], bf16, tag="zb")
            yt = cpool.tile([H, BS], f32, tag="yt")         # Y^T accumulator
            nc.vector.memset(yt[:], 0.0)

            # Early dummy collective: absorbs cross-core launch skew while
            # the engines do setup + rank-0 compute (collectives run on
            # TOPSP/SDMA, serialized before the first real sync).
            dsrc = dpool.tile([H, 2], f32, tag="ccsrc")
            ddst = dpool.tile([N_CORES * H, 2], f32, tag="ccdst")
            nc.sync.dma_start(dsrc[:], bn[:, 0:2])
            nc.gpsimd.collective_compute(
                "AllGather", AL.bypass, replica_groups=rg,
                ins=[dsrc.opt()], outs=[ddst.opt()],
            )

            # ---------------- Z = relu(X@W1 + b1) ----------------
            for bt in range(NBT):
                ps = psmm.tile([128, MACRO], f32, tag="mm")
                for c in range(2):
                    nc.tensor.matmul(
                        ps[:, :H],
                        lhsT=xt[:, c * BS + bt * 128: c * BS + (bt + 1) * 128],
                        rhs=w1[:, c * H:(c + 1) * H],
                        start=(c == 0), stop=(c == 1),
                    )
                t0 = spool.tile([128, H], f32, tag="ztmp")
                nc.vector.tensor_tensor(t0[:], ps[:, :H], b1b[:], AL.add)
                nc.scalar.activation(zf[:, bt * H:(bt + 1) * H], t0[:], FT.Relu)
                nc.vector.tensor_copy(zb[:, bt * H:(bt + 1) * H],
                                      zf[:, bt * H:(bt + 1) * H])

            # Z^T (q-part, b) = initial Zi^T
            zit = zitpool.tile([H, BS], bf16, tag="zit")
            for bt in range(NBT):
                pst = pstr.tile([128, 128], bf16, tag="tr")
                nc.tensor.transpose(pst[:],
                                    zb[:, bt * H:(bt + 1) * H], ident[:])
                nc.scalar.activation(zit[:, bt * 128:(bt + 1) * 128],
                                     pst[:], FT.Copy)

            # ---------------- scan over ranks ----------------
            for r in range(RANK):
                p_sb = ppool.tile([128, QK], bf16, tag="p")
                nc.sync.dma_start(p_sb[:], Pd[r])

                gbf = spool.tile([128, NBT * H], bf16, tag="gbf")
                for bt in range(NBT):
                    tmp = tmpool.tile([128, QK], bf16, tag="tmp")
                    acc = psacc.tile([128, 512], f32, tag="acc")
                    lhs = zit[:, bt * 128:(bt + 1) * 128]
                    nhalf = 2 * NCHUNK  # 512-wide id-MM count
                    for c in range(NCHUNK):
                        # stage-1 matmuls: two 512-wide into one macro psum
                        ps = psmm.tile([128, MACRO], f32, tag="mm")
                        for h in range(MACRO // 512):
                            nc.tensor.matmul(
                                ps[:, h * 512:(h + 1) * 512], lhsT=lhs,
                                rhs=p_sb[:, c * MACRO + h * 512:
                                         c * MACRO + (h + 1) * 512],
                                start=True, stop=True)
                        # scale all QPM q-planes in one DVE op:
                        # tmp[b, q, k] = psum[b, q, k] * Z[b, q]
                        zsl = zf[:, bt * H + c * QPM: bt * H + (c + 1) * QPM]
                        nc.vector.tensor_tensor(
                            tmp[:, c * MACRO:(c + 1) * MACRO].rearrange(
                                "p (a b) -> p a b", b=H),
                            ps[:].rearrange("p (a b) -> p a b", b=H),
                            zsl.broadcast_to((128, QPM, H)),
                            AL.mult)
                        # identity-matmul accumulation: 4 planes per MM into
                        # a 512-wide accumulator (folded 4->1 afterwards)
                        for h in range(2):
                            hi = 2 * c + h
                            nc.tensor.matmul(
                                acc[:], lhsT=ident[:],
                                rhs=tmp[:, hi * 512:(hi + 1) * 512],
                                start=(hi == 0), stop=(hi == nhalf - 1),
                            )
                    # fold 4 accumulator slots -> G, evac bf16 for transpose
                    f4 = spool.tile([128, 512], f32, tag="fold4")
                    nc.vector.tensor_copy(f4[:], acc[:])
                    f2 = spool.tile([128, 256], f32, tag="fold2")
                    nc.vector.tensor_tensor(f2[:], f4[:, 0:256],
                                            f4[:, 256:512], AL.add)
                    f1 = spool.tile([128, 128], f32, tag="fold1")
                    nc.vector.tensor_tensor(f1[:], f2[:, 0:128],
                                            f2[:, 128:256], AL.add)
                    nc.vector.tensor_copy(gbf[:, bt * H:(bt + 1) * H], f1[:])

                # transpose G -> (k, b), evac + batch stats via accum_out
                gt = spool.tile([H, BS], bf16, tag="gt")
                scr = spool.tile([128, 128], bf16, tag="scr")
                s1 = spool.tile([H, 8], f32, tag="stat")
                for bt in range(NBT):
                    pst = pstr.tile([128, 128], bf16, tag="tr")
                    nc.tensor.transpose(pst[:],
                                        gbf[:, bt * H:(bt + 1) * H], ident[:])
                    nc.scalar.activation(gt[:, bt * 128:(bt + 1) * 128],
                                         pst[:], FT.Copy,
                                         accum_out=s1[:, bt:bt + 1])
                    nc.scalar.activation(scr[:], pst[:], FT.Square,
                                         accum_out=s1[:, 2 + bt:3 + bt])
                stl = spool.tile([H, 2], f32, tag="stl")
                nc.vector.tensor_tensor(stl[:, 0:1], s1[:, 0:1], s1[:, 1:2],
                                        AL.add)
                nc.vector.tensor_tensor(stl[:, 1:2], s1[:, 2:3], s1[:, 3:4],
                                        AL.add)

                # ---- cross-core AllGather of (H, 2) stats ----
                a_ap, c_ap = _bn_sync(nc, tc, dpool, spool, stl, bn,
                                      gcol=0, bcol=1, extra_scale=None)

                # apply BN + produce next Zi^T; accumulate Y^T
                zit_next = zitpool.tile([H, BS], bf16, tag="zit")
                nc.vector.tensor_scalar(zit_next[:], gt[:], a_ap, c_ap,
                                        AL.mult, AL.add)
                nc.vector.tensor_tensor(yt[:], yt[:], zit_next[:], AL.add)
                zit = zit_next

            # ---------------- Y BN (on Y/8 via stats scale trick) ----------
            sy = spool.tile([H, 8], f32, tag="stat")
            scr2 = spool.tile([H, BS], bf16, tag="scry")
            nc.scalar.activation(scr2[:], yt[:], FT.Copy, scale=0.125,
                                 accum_out=sy[:, 0:1])
            nc.scalar.activation(scr2[:], yt[:], FT.Square, scale=0.125,
                                 accum_out=sy[:, 1:2])
            styl = spool.tile([H, 2], f32, tag="stl")
            nc.vector.tensor_copy(styl[:], sy[:, 0:2])
            ay_ap, cy_ap = _bn_sync(nc, tc, dpool, spool, styl, bn,
                                    gcol=2, bcol=3, extra_scale=0.125)
            ybn = spool.tile([H, BS], bf16, tag="ybn")
            nc.vector.tensor_scalar(ybn[:], yt[:], ay_ap, cy_ap,
                                    AL.mult, AL.add)

            # ---------------- final: relu(relu(Y@W3+b3)+relu(X@W2+b2)) ----
            for bt in range(NBT):
                psA = psmm.tile([128, MACRO], f32, tag="mm")
                nc.tensor.matmul(psA[:, :OUT],
                                 lhsT=ybn[:, bt * 128:(bt + 1) * 128],
                                 rhs=w3[:], start=True, stop=True)
                r1 = spool.tile([128, OUT], f32, tag="r1")
                nc.vector.tensor_tensor(r1[:], psA[:, :OUT], b3b[:], AL.add)
                r1r = spool.tile([128, OUT], f32, tag="r1r")
                nc.scalar.activation(r1r[:], r1[:], FT.Relu)

                psB = psmm.tile([128, MACRO], f32, tag="mm")
                for c in range(2):
                    nc.tensor.matmul(
                        psB[:, :OUT],
                        lhsT=xt[:, c * BS + bt * 128: c * BS + (bt + 1) * 128],
                        rhs=w2[:, c * OUT:(c + 1) * OUT],
                        start=(c == 0), stop=(c == 1),
                    )
                r2 = spool.tile([128, OUT], f32, tag="r2")
                nc.vector.tensor_tensor(r2[:], psB[:, :OUT], b2b[:], AL.add)
                r2r = spool.tile([128, OUT], f32, tag="r2r")
                nc.scalar.activation(r2r[:], r2[:], FT.Relu)

                s = spool.tile([128, OUT], f32, tag="s")
                nc.vector.tensor_tensor(s[:], r1r[:], r2r[:], AL.add)
                of = spool.tile([128, OUT], f32, tag="of")
                nc.scalar.activation(of[:], s[:], FT.Relu)
                nc.sync.dma_start(OUTd[bt * 128:(bt + 1) * 128, :], of[:])

    nc.compile()
    return nc


def _bn_sync(nc, tc, dpool, spool, stl, bn, gcol, bcol, extra_scale):
    """AllGather per-core (H,2) [sum, sumsq] stats, reduce across 8 cores,
    compute affine coeffs a, c s.t. BN(x) = a*x + c (per-partition).

    If extra_scale is set, stats were computed on (extra_scale*x) and the
    returned a is pre-multiplied by extra_scale so a*x + c uses raw x.
    """
    from concourse import mybir

    f32 = mybir.dt.float32
    FT = mybir.ActivationFunctionType
    AL = mybir.AluOpType

    src = dpool.tile([H, 2], f32, tag="ccsrc")
    dst = dpool.tile([N_CORES * H, 2], f32, tag="ccdst")
    nc.sync.dma_start(src[:], stl[:])
    nc.gpsimd.collective_compute(
        "AllGather", AL.bypass, replica_groups=[list(range(N_CORES))],
        ins=[src.opt()], outs=[dst.opt()],
    )
    gath = spool.tile([H, 16], f32, tag="gath")
    nc.sync.dma_start(
        gath[:].rearrange("k (c s) -> k c s", c=N_CORES),
        dst[:].rearrange("(c k) s -> k c s", c=N_CORES))
    # reduce over cores: layout (k, (c, s)) c-major pairs
    r4 = spool.tile([H, 8], f32, tag="r4")
    nc.vector.tensor_tensor(r4[:], gath[:, 0:8], gath[:, 8:16], AL.add)
    r2 = spool.tile([H, 4], f32, tag="r2s")
    nc.vector.tensor_tensor(r2[:], r4[:, 0:4], r4[:, 4:8], AL.add)
    st = spool.tile([H, 2], f32, tag="stg")
    nc.vector.tensor_tensor(st[:], r2[:, 0:2], r2[:, 2:4], AL.add)

    cf = spool.tile([H, 8], f32, tag="cf")
    m = cf[:, 0:1]
    ex2 = cf[:, 1:2]
    v = cf[:, 2:3]
    sd = cf[:, 3:4]
    rinv = cf[:, 4:5]
    a = cf[:, 5:6]
    t = cf[:, 6:7]
    c = cf[:, 7:8]
    nc.vector.tensor_scalar(m, st[:, 0:1], 1.0 / B, None, AL.mult)
    nc.vector.tensor_scalar(ex2, st[:, 1:2], 1.0 / B, None, AL.mult)
    msq = spool.tile([H, 1], f32, tag="msq")
    nc.vector.tensor_tensor(msq[:], m, m, AL.mult)
    nc.vector.tensor_tensor(v, ex2, msq[:], AL.subtract)
    nc.vector.tensor_scalar(v, v, EPS, None, AL.add)
    nc.scalar.activation(sd, v, FT.Sqrt)
    nc.vector.reciprocal(rinv, sd)
    nc.vector.tensor_tensor(a, rinv, bn[:, gcol:gcol + 1], AL.mult)
    nc.vector.tensor_tensor(t, m, a, AL.mult)
    nc.vector.tensor_tensor(c, bn[:, bcol:bcol + 1], t, AL.subtract)
    if extra_scale is not None:
        a_out = cf[:, 4:5]  # reuse rinv slot
        nc.vector.tensor_scalar(a_out, a, extra_scale, None, AL.mult)
        return a_out, c
    return a, c


def _prep_inputs(X, W1, b1, W2, b2, W3, b3, P, gz, bz, gy, by):
    bf = ml_dtypes.bfloat16
    per_core = []
    P_b = np.ascontiguousarray(P.reshape(RANK, H, QK)).astype(bf)
    W1_b = np.ascontiguousarray(W1.reshape(2, 128, H)).astype(bf)
    W2_b = np.ascontiguousarray(W2.reshape(2, 128, OUT)).astype(bf)
    W3_b = np.ascontiguousarray(W3).astype(bf)
    b1b = np.broadcast_to(b1, (128, H)).astype(np.float32).copy()
    b2b = np.broadcast_to(b2, (128, OUT)).astype(np.float32).copy()
    b3b = np.broadcast_to(b3, (128, OUT)).astype(np.float32).copy()
    bnc = np.stack([gz, bz, gy, by], axis=1).astype(np.float32)
    ident = np.eye(128, dtype=np.float32).astype(bf)
    for s in range(N_CORES):
        Xs = X[s * BS:(s + 1) * BS]
        XT = np.ascontiguousarray(Xs.T.reshape(2, 128, BS)).astype(bf)
        per_core.append({
            "XT": XT, "P": P_b, "W1": W1_b, "W2": W2_b, "W3": W3_b,
            "b1b": b1b, "b2b": b2b, "b3b": b3b, "bn": bnc, "ident": ident,
        })
    return per_core


def kernel(**inputs):
    _ensure_axon_hooks_shim()
    from concourse.bass_utils import run_bass_kernel_spmd

    if "nc" not in _cache:
        _cache["nc"] = _build()
    nc = _cache["nc"]

    in_maps = _prep_inputs(**{k: np.asarray(v) for k, v in inputs.items()})
    res = run_bass_kernel_spmd(nc, in_maps, core_ids=list(range(N_CORES)))
    out = np.concatenate([m["out"] for m in res.results], axis=0)
    return out.astype(np.float32)


if __name__ == "__main__":
    import reference as R

    inputs = {k: np.asarray(v) for k, v in R.setup_inputs().items()}
    got = kernel(**inputs)
    exp = np.asarray(R.reference(**R.setup_inputs()))
    rel = np.linalg.norm(got - exp) / np.linalg.norm(exp)
    print("rel l2:", rel)
